# revision 32
# baseline (speedup 1.0000x reference)
"""Trainium2 Bass kernel for nn_BatchedTeacherPolicy.

2048 independent per-teacher MLPs (obs-norm -> 48->512->256->128->12,
ELU between layers, tanh at the end). Pure data parallel: 256 teachers
per NeuronCore across 8 cores, 2 groups of 128 teachers per core.

The kernel is HBM-bound (358 GB/s per-core share), so bytes are the
primary lever, then keeping the DMA queues and the PE continuously fed:
- W1 (69% of weight traffic) is stored as fp8 E3M4 (float8e3) with a
  global scale s1 (sigma -> 2); the 1/s1 dequant is folded into the L1
  epilogue's bias-add, and the PE consumes fp8 stationary weights
  directly against fp16 moving activations. Everything else is fp16
  (same bytes as bf16, 8x less rounding noise). Measured budget:
  fp16-everything 6.1e-4, +W1-fp8 1.42e-2 vs the 2e-2 gate; fp8 on a
  second layer would bust it (2.0e-2).
- L1/L2/L3 run on the TensorEngine in column space (activations as
  [feature, teacher]); each teacher's matvec is a stationary-weight
  LDWEIGHTS + 1-column MATMUL pair accumulating into [O, teachers]
  PSUM tiles. The ELU(y)+1 = exp(min(y,0)) + max(y,0) trick folds the
  "-1" into the next layer's bias on the host (b' = b - W_quant @ 1).
- L0 is split across engines to shorten the pipeline head: group 0
  computes L0 on the PE in column space (W0 g0 is host-transposed to
  [i, t, o]; its output IS x1T), so the big col-weight stream starts
  ~35us in; group 1 computes L0 on the otherwise-idle DVE as i-major
  fp16 FMA chains, hidden under group 0's col phase.
- DMA ordering: the col-weight ring (gpsimd-issued) carries W0c g0
  first, then W1/W2/W3 in exact PE consumption order. Ring-priority is
  enforced with pool-buffer WAW gates (dummy 1-element writes gated on
  W0c g0's last tile) because the tile scheduler is free to reorder
  anything without a data dependency.
"""

from contextlib import ExitStack

import numpy as np
import ml_dtypes

import concourse.bass as bass
import concourse.bacc as bacc
import concourse.tile as tile
from concourse import mybir
from concourse.bass_utils import run_bass_kernel_spmd

N, OBS = 2048, 48
DIMS = [(512, 48), (256, 512), (128, 256), (12, 128)]  # (out, in) per layer
N_CORES = 8
NPC = N // N_CORES  # teachers per core
P = 128             # partitions = teachers per group
G = NPC // P        # groups per core (2)

O0 = DIMS[0][0]          # 512
O1, I1 = DIMS[1]         # 256, 512
O2, I2 = DIMS[2]         # 128, 256
O3, I3 = DIMS[3]         # 12, 128
CI1 = I1 // P            # 4 contraction chunks for L1
OC1 = O1 // P            # 2 output chunks for L1
CI2 = I2 // P            # 2 contraction chunks for L2
OC0 = O0 // P            # 4 output chunks for L0 (PE path)
TS1 = 32                 # teachers per W1 DMA tile (8KB/partition in fp8)
TS2 = 32                 # teachers per W2 DMA tile (8KB/partition in fp16)
TB0 = 16                 # teachers per W0c DMA tile (PE L0 path, 16KB lines)
OCH0 = 256               # L0 output chunk (DVE FMA path, group 1)
NCH0 = O0 // OCH0        # 2 chunks

# W1 fp8 quantization scale: sigma(W1) = 1/sqrt(512) -> 2.0. Absmax of
# W1*s1 is ~12, inside E3M4's +-15.5 normal range; host clips to +-15.
S1 = 2.0 * np.sqrt(512.0)
INV_S1 = float(1.0 / S1)

F32 = mybir.dt.float32
F16 = mybir.dt.float16
FP8 = mybir.dt.float8e3
AF = mybir.ActivationFunctionType
ALU = mybir.AluOpType
NPF16 = np.float16
NPFP8 = ml_dtypes.float8_e3m4

_cached = {}


def _build_bass():
    nc = bacc.Bacc(trn_type="TRN2", target_bir_lowering=False)

    # host-packed [obs, -mean, 1/std] along dim 1
    nrm_d = nc.dram_tensor("nrm", [NPC, 3, OBS], F16, kind="ExternalInput")
    # group 0: W0 col-space [i, t, o] for the PE L0 path; b0 as [o, oc, t]
    w0c_d = nc.dram_tensor("W0C", [OBS, P, O0], F16, kind="ExternalInput")
    b0c_d = nc.dram_tensor("b0C", [P, OC0, P], F16, kind="ExternalInput")
    # group 1: W0 row-space i-major o-chunked [t, ch, i, o] for the DVE path
    w0r_d = nc.dram_tensor("W0R", [P, NCH0, OBS, OCH0], F16, kind="ExternalInput")
    b0r_d = nc.dram_tensor("b0R", [P, O0], F16, kind="ExternalInput")
    # col-space stacked weights: [g, ci, i_local(part), teacher, o]
    w1_d = nc.dram_tensor("W1T", [G, CI1, P, P, O1], FP8, kind="ExternalInput")
    b1_d = nc.dram_tensor("b1T", [G, OC1, P, P], F16, kind="ExternalInput")
    w2_d = nc.dram_tensor("W2T", [G, CI2, P, P, O2], F16, kind="ExternalInput")
    b2_d = nc.dram_tensor("b2T", [G, P, P], F16, kind="ExternalInput")
    w3_d = nc.dram_tensor("W3T", [G, P, P, O3], F16, kind="ExternalInput")
    b3_d = nc.dram_tensor("b3T", [G, O3, P], F16, kind="ExternalInput")
    # col-space output [o, t]: 12 fat descriptors per group; the host
    # transposes at gather time
    out_d = nc.dram_tensor("out", [O3, NPC], F32, kind="ExternalOutput")

    from concourse.masks import make_identity

    with ExitStack() as ctx:
        tc = ctx.enter_context(tile.TileContext(nc))
        w0cpool = ctx.enter_context(tc.tile_pool(name="w0cpool", bufs=2))
        w0pool = ctx.enter_context(tc.tile_pool(name="w0pool", bufs=2))
        wcpool = ctx.enter_context(tc.tile_pool(name="wcpool", bufs=10))
        w3pool = ctx.enter_context(tc.tile_pool(name="w3pool", bufs=2))
        xpool = ctx.enter_context(tc.tile_pool(name="xpool", bufs=2))
        spool = ctx.enter_context(tc.tile_pool(name="spool", bufs=3))
        bpool = ctx.enter_context(tc.tile_pool(name="bpool", bufs=2))
        ppool = ctx.enter_context(tc.tile_pool(name="ppool", bufs=1, space="PSUM"))
        ipool = ctx.enter_context(tc.tile_pool(name="ipool", bufs=1))

        ident_h = ipool.tile([P, P], F16)
        make_identity(nc, ident_h)

        def emit_rowA(g):
            """Ring A (scalar-issued) DMAs for group g: norm inputs and
            biases; for group 1 also the row-space W0 chunks."""
            n0 = g * P
            nrm_t = spool.tile([P, 3, OBS], F16, tag="nrm", bufs=2, name=f"nrm_{g}")
            nc.scalar.dma_start(out=nrm_t, in_=nrm_d[n0 : n0 + P, :, :])
            b1t = bpool.tile([P, OC1, P], F16, tag="b1", name=f"b1_{g}")
            for oc in range(OC1):
                nc.scalar.dma_start(out=b1t[:, oc, :], in_=b1_d[g, oc])
            b2t = bpool.tile([P, P], F16, tag="b2", name=f"b2_{g}")
            nc.scalar.dma_start(out=b2t, in_=b2_d[g])
            b3t = bpool.tile([O3, P], F16, tag="b3", name=f"b3_{g}")
            nc.scalar.dma_start(out=b3t, in_=b3_d[g])
            if g == 0:
                b0t = bpool.tile([P, OC0, P], F16, tag="b0c", bufs=1, name="b0c_0")
                nc.scalar.dma_start(out=b0t, in_=b0c_d[:, :, :])
                w0ts = None
            else:
                b0t = bpool.tile([P, O0], F16, tag="b0r", bufs=1, name="b0r_1")
                nc.scalar.dma_start(out=b0t, in_=b0r_d[:, :])
                w0ts = []
                for ch in range(NCH0):
                    wt = w0pool.tile([P, OBS, OCH0], F16, tag="w0", name=f"w0_1_{ch}")
                    nc.scalar.dma_start(out=wt, in_=w0r_d[:, ch])
                    w0ts.append(wt)
            return nrm_t, b0t, b1t, b2t, b3t, w0ts

        def emit_w0c_dmas():
            """Ring B head: group 0's col-space W0 stream (full priority)."""
            w0cts = []
            for tb in range(0, P, TB0):
                wt = w0cpool.tile([OBS, TB0, O0], F16, tag="w0c", name=f"w0c_{tb}")
                nc.gpsimd.dma_start(out=wt, in_=w0c_d[:, tb : tb + TB0, :])
                w0cts.append(wt)
            return w0cts

        def emit_gates(w0c_last, w0r_tiles_pending):
            """Pool-buffer WAW gates: delay (a) ring A's W0 g1 chunks and
            (b) ring B's W1/W2 stream until W0c g0 has fully landed, so the
            head of the kernel gives group 0's weights the whole pipe."""
            src = w0c_last[:1, 0, :1]
            if w0r_tiles_pending:
                for k in range(2):
                    d = w0pool.tile([P, OBS, OCH0], F16, tag="w0", name=f"gA_{k}")
                    nc.gpsimd.tensor_copy(d[:1, 0, :1], src)
            for k in range(10):
                d = wcpool.tile([P, TS1 // 2, O1], F16, tag="wc", name=f"gB_{k}")
                nc.gpsimd.tensor_copy(d[:1, 0, :1], src)

        def norm_x0(g, nrm_t):
            """x0 = clip((obs-mean)/std, -5, 5) on DVE."""
            x0 = spool.tile([P, OBS], F32, tag="x0", name=f"x0_{g}")
            nc.vector.tensor_add(x0, nrm_t[:, 0, :], nrm_t[:, 1, :])
            nc.vector.tensor_mul(x0, x0, nrm_t[:, 2, :])
            x0h = xpool.tile([P, OBS], F32, tag="x0h", name=f"x0h_{g}")
            nc.vector.tensor_scalar(
                out=x0h, in0=x0, scalar1=-5.0, scalar2=5.0,
                op0=ALU.max, op1=ALU.min,
            )
            x0f = xpool.tile([P, OBS], F16, tag="x0f", name=f"x0f_{g}")
            nc.vector.tensor_copy(x0f, x0h)
            return x0h, x0f

        def elu_chunk(yps, babs, out_ap, g, li, oc, scale=None):
            """out = exp(min(y,0)) + max(y,0) with y = scale*psum + bias."""
            t0 = spool.tile([P, P], F32, tag="t0", name=f"t0_{g}_{li}_{oc}")
            if scale is None:
                nc.vector.tensor_add(t0, yps, babs)
            else:
                nc.vector.scalar_tensor_tensor(
                    out=t0, in0=yps, scalar=scale, in1=babs,
                    op0=ALU.mult, op1=ALU.add,
                )
            e = spool.tile([P, P], F32, tag="el", name=f"el_{g}_{li}_{oc}")
            nc.vector.tensor_scalar_min(e, t0, 0.0)
            nc.scalar.activation(e, e, AF.Exp)
            nc.vector.scalar_tensor_tensor(
                out=out_ap, in0=t0, scalar=0.0, in1=e, op0=ALU.max, op1=ALU.add,
            )

        def emit_babs(g, b1t, b2t, b3t):
            """Absorb the bias-DMA waits ahead of the col-phase epilogues."""
            b1a = bpool.tile([P, OC1, P], F32, tag="b1a", name=f"b1a_{g}")
            nc.vector.tensor_scalar_mul(b1a, b1t, 1.0)
            b2a = bpool.tile([P, P], F32, tag="b2a", name=f"b2a_{g}")
            nc.vector.tensor_scalar_mul(b2a, b2t, 1.0)
            b3a = bpool.tile([O3, P], F32, tag="b3a", name=f"b3a_{g}")
            nc.vector.tensor_scalar_mul(b3a, b3t, 1.0)
            return b1a, b2a, b3a

        def emit_l0_pe(w0cts, x0f, b0t):
            """Group 0 L0 on the PE: x0T = transpose(x0), then per teacher
            4 stationary [48,128] W0 chunks x 1-column matmuls into
            [128 o, 128 t] PSUM tiles; ELU lands directly in x1T layout."""
            pst0 = ppool.tile([OBS, P], F16, tag="pst", bufs=2, name="pst0")
            nc.tensor.transpose(pst0, x0f, ident_h)
            x0T = xpool.tile([OBS, P], F16, tag="x0T", name="x0T_0")
            nc.vector.tensor_copy(x0T, pst0)

            # all 4 output chunks in ONE psum tile (exactly one 2KB bank)
            yps0 = ppool.tile([P, OC0, P], F32, tag="yps0", name="yps0")
            for tb_i, tb in enumerate(range(0, P, TB0)):
                wt = w0cts[tb_i]
                for tl in range(TB0):
                    t = tb + tl
                    for oc in range(OC0):
                        nc.tensor.matmul(
                            yps0[:, oc, t : t + 1],
                            lhsT=wt[:, tl, oc * P : (oc + 1) * P],
                            rhs=x0T[:, t : t + 1],
                            start=True,
                            stop=True,
                        )
            x1T = xpool.tile([P, CI1, P], F16, tag="x1T", name="x1T_0")
            for oc in range(OC0):
                elu_chunk(yps0[:, oc, :], b0t[:, oc, :], x1T[:, oc, :], 0, 0, oc)
            return x1T

        def emit_l0_dve(g, nrm_t, b0t, w0ts):
            """Group 1 L0 on DVE: i-major fp16 FMA chains (4 interleaved),
            then ELU to x1h in row space."""
            x0h, x0f = norm_x0(g, nrm_t)
            HI = OBS // 2
            y0 = xpool.tile([P, O0], F16, tag="y0", name=f"y0_{g}")
            y0b = xpool.tile([P, O0], F16, tag="y0b", name=f"y0b_{g}")
            for i in range(HI):
                for ch in range(NCH0):
                    c0 = ch * OCH0
                    for acc, ioff in ((y0, 0), (y0b, HI)):
                        ysl = acc[:, c0 : c0 + OCH0]
                        if i == 0 and ioff != 0:
                            nc.vector.tensor_scalar_mul(
                                ysl, w0ts[ch][:, ioff, :],
                                x0h[:, ioff : ioff + 1],
                            )
                        else:
                            nc.vector.scalar_tensor_tensor(
                                out=ysl,
                                in0=w0ts[ch][:, i + ioff, :],
                                scalar=x0f[:, i + ioff : i + ioff + 1],
                                in1=b0t[:, c0 : c0 + OCH0] if i == 0 else ysl,
                                op0=ALU.mult,
                                op1=ALU.add,
                            )
            nc.vector.tensor_add(y0, y0, y0b)
            e0 = spool.tile([P, O0], F32, tag="e0", name=f"e0_{g}")
            nc.vector.tensor_scalar_min(e0, y0, 0.0)
            nc.scalar.activation(e0, e0, AF.Exp)
            x1h = xpool.tile([P, O0], F16, tag="x1h", name=f"x1h_{g}")
            nc.vector.scalar_tensor_tensor(
                out=x1h, in0=y0, scalar=0.0, in1=e0, op0=ALU.max, op1=ALU.add,
            )
            return x1h

        def emit_x1prep(g, x1h):
            """x1' [t, 512] -> x1T [ci][i, t] (fp16) via PE transposes."""
            x1T = xpool.tile([P, CI1, P], F16, tag="x1Tb", name=f"x1T_{g}")
            for ci in range(CI1):
                pst = ppool.tile([P, P], F16, tag="pst", bufs=2, name=f"pst_{g}_{ci}")
                nc.tensor.transpose(pst, x1h[:, ci * P : (ci + 1) * P], ident_h)
                nc.vector.tensor_copy(x1T[:, ci, :], pst)
            return x1T

        def emit_col(g, x1T, b1a, b2a, b3a):
            """Column-space phase on PE: L1/L2/L3 as per-teacher
            LDWEIGHTS+MATMUL pairs, epilogues on DVE/ACT."""
            n0 = g * P

            # ---- L1 (fp8 weights) ----
            yps1 = [
                ppool.tile([P, P], F32, tag=f"yps1_{oc}", name=f"yps1_{g}_{oc}")
                for oc in range(OC1)
            ]
            for tb in range(0, P, TS1):
                wts = []
                for ci in range(CI1):
                    wt = wcpool.tile(
                        [P, TS1, O1], FP8, tag="wc", name=f"w1_{g}_{tb}_{ci}"
                    )
                    nc.gpsimd.dma_start(out=wt, in_=w1_d[g, ci, :, tb : tb + TS1, :])
                    wts.append(wt)
                for tl in range(TS1):
                    t = tb + tl
                    for oc in range(OC1):
                        for ci in range(CI1):
                            nc.tensor.matmul(
                                yps1[oc][:, t : t + 1],
                                lhsT=wts[ci][:, tl, oc * P : (oc + 1) * P],
                                rhs=x1T[:, ci, t : t + 1],
                                start=(ci == 0),
                                stop=(ci == CI1 - 1),
                            )
            x2T = xpool.tile([P, CI2, P], F16, tag="x2T", name=f"x2T_{g}")
            for oc in range(OC1):
                elu_chunk(yps1[oc], b1a[:, oc, :], x2T[:, oc, :], g, 1, oc,
                          scale=INV_S1)

            # W3T is tiny and lives in its own pool: issue its DMA ahead of
            # the W2T stream so it never lands in the tail.
            w3t = w3pool.tile([P, P, O3], F16, tag="w3", name=f"w3_{g}")
            nc.gpsimd.dma_start(out=w3t, in_=w3_d[g])
            # ---- L2 ----
            yps2 = ppool.tile([P, P], F32, tag="yps2", name=f"yps2_{g}")
            for tb in range(0, P, TS2):
                wts = []
                for ci in range(CI2):
                    wt = wcpool.tile(
                        [P, TS2, O2], F16, tag="wc", name=f"w2_{g}_{tb}_{ci}"
                    )
                    nc.gpsimd.dma_start(out=wt, in_=w2_d[g, ci, :, tb : tb + TS2, :])
                    wts.append(wt)
                for tl in range(TS2):
                    t = tb + tl
                    for ci in range(CI2):
                        nc.tensor.matmul(
                            yps2[:, t : t + 1],
                            lhsT=wts[ci][:, tl, :],
                            rhs=x2T[:, ci, t : t + 1],
                            start=(ci == 0),
                            stop=(ci == CI2 - 1),
                        )
            x3T = xpool.tile([P, P], F16, tag="x3T", name=f"x3T_{g}")
            elu_chunk(yps2, b2a, x3T, g, 2, 0)

            # ---- L3 ----
            yps3 = ppool.tile([O3, P], F32, tag="yps3", name=f"yps3_{g}")
            for t in range(P):
                nc.tensor.matmul(
                    yps3[:, t : t + 1],
                    lhsT=w3t[:, t, :],
                    rhs=x3T[:, t : t + 1],
                    start=True,
                    stop=True,
                )
            y3 = spool.tile([O3, P], F32, tag="y3", name=f"y3_{g}")
            nc.vector.tensor_add(y3, yps3, b3a)
            nc.scalar.activation(y3, y3, AF.Tanh)
            nc.scalar.dma_start(out=out_d[:, n0 : n0 + P], in_=y3)

        # ---- emission ----
        rowA0 = emit_rowA(0)
        w0cts = emit_w0c_dmas()
        rowA1_pregate = True
        emit_gates(w0cts[-1], rowA1_pregate)
        rowA1 = emit_rowA(1)

        _, x0f0 = norm_x0(0, rowA0[0])
        x1T0 = emit_l0_pe(w0cts, x0f0, rowA0[1])
        babs0 = emit_babs(0, rowA0[2], rowA0[3], rowA0[4])
        # group 1's DVE work is emitted BEFORE col0 so its FMA chain is not
        # stuck behind col0's PSUM-gated epilogues in the in-order DVE queue
        x1h1 = emit_l0_dve(1, rowA1[0], rowA1[1], rowA1[5])
        babs1 = emit_babs(1, rowA1[2], rowA1[3], rowA1[4])
        emit_col(0, x1T0, *babs0)
        x1T1 = emit_x1prep(1, x1h1)
        emit_col(1, x1T1, *babs1)

    nc.compile()
    return nc


def _get_nc():
    if "nc" not in _cached:
        _cached["nc"] = _build_bass()
    return _cached["nc"]


def _pack_core_inputs(full, c):
    """Shard + lay out one core's inputs (fp8 W1, fp16 rest)."""
    sl = slice(c * NPC, (c + 1) * NPC)
    f32 = np.float32
    w0 = np.asarray(full["W0"])[sl].astype(NPF16)           # [NPC, 512, 48]
    # group 0: col-space [i, t, o]
    w0c = np.ascontiguousarray(w0[:P].transpose(2, 0, 1))
    # group 1: row-space i-major o-chunked [t, ch, i, o]
    w0r = np.ascontiguousarray(
        w0[P:].transpose(0, 2, 1).reshape(P, OBS, NCH0, OCH0).transpose(0, 2, 1, 3)
    )
    b0 = np.asarray(full["b0"])[sl].astype(NPF16)
    b0c = np.ascontiguousarray(b0[:P].reshape(P, OC0, P).transpose(2, 1, 0))
    b0r = np.ascontiguousarray(b0[P:])
    w1f = np.asarray(full["W1"])[sl].astype(f32)            # [NPC, 256, 512]
    w1q = np.clip(w1f * f32(S1), -15.0, 15.0).astype(NPFP8)
    w1deq = w1q.astype(f32) * f32(INV_S1)
    w2 = np.asarray(full["W2"])[sl].astype(NPF16)           # [NPC, 128, 256]
    w3 = np.asarray(full["W3"])[sl].astype(NPF16)           # [NPC, 12, 128]
    # fold the ELU "-1" into the next layer's bias: b' = b - W_quant @ 1
    b1p = np.asarray(full["b1"])[sl] - w1deq.sum(-1)
    b2p = np.asarray(full["b2"])[sl] - w2.astype(f32).sum(-1)
    b3p = np.asarray(full["b3"])[sl] - w3.astype(f32).sum(-1)
    # W1T[g, ci, i, t, oc*128+o] = W1[g*128+t, oc*128+o, ci*128+i]
    w1t = np.ascontiguousarray(
        w1q.reshape(G, P, OC1, P, CI1, P).transpose(0, 4, 5, 1, 2, 3)
        .reshape(G, CI1, P, P, O1)
    )
    b1t = np.ascontiguousarray(
        b1p.reshape(G, P, OC1, P).transpose(0, 2, 3, 1).astype(NPF16)
    )
    w2t = np.ascontiguousarray(
        w2.reshape(G, P, P, CI2, P).transpose(0, 3, 4, 1, 2)
    )
    b2t = np.ascontiguousarray(b2p.reshape(G, P, P).transpose(0, 2, 1).astype(NPF16))
    w3t = np.ascontiguousarray(w3.reshape(G, P, O3, P).transpose(0, 3, 1, 2))
    b3t = np.ascontiguousarray(b3p.reshape(G, P, O3).transpose(0, 2, 1).astype(NPF16))
    nrm = np.stack(
        [
            np.asarray(full["obs"])[sl],
            -np.asarray(full["mean"])[sl],
            1.0 / np.asarray(full["std"])[sl],
        ],
        axis=1,
    ).astype(NPF16)
    return {
        "nrm": np.ascontiguousarray(nrm),
        "W0C": w0c, "b0C": b0c,
        "W0R": w0r, "b0R": b0r,
        "W1T": w1t, "b1T": b1t,
        "W2T": w2t, "b2T": b2t,
        "W3T": w3t, "b3T": b3t,
    }


def kernel(obs, mean, std, W0, b0, W1, b1, W2, b2, W3, b3, _trace=False):
    nc = _get_nc()
    full = {
        "obs": obs, "mean": mean, "std": std,
        "W0": W0, "b0": b0, "W1": W1, "b1": b1,
        "W2": W2, "b2": b2, "W3": W3, "b3": b3,
    }
    in_maps = [_pack_core_inputs(full, c) for c in range(N_CORES)]
    res = run_bass_kernel_spmd(
        nc, in_maps, core_ids=list(range(N_CORES)), trace=_trace
    )
    _cached["last_results"] = res
    out = np.concatenate(
        [np.ascontiguousarray(res.results[c]["out"].T) for c in range(N_CORES)],
        axis=0,
    )
    return out
